# revision 1
# baseline (speedup 1.0000x reference)
"""Trainium2 Bass kernel for nn_ASPPConv (gated dilated conv + BatchNorm + ReLU).

Reference computation (per batch element b):
    for k in 0..9:  out[b] += W[:,:,k] @ (x_shift_k[b] * g_k[b])
    g_k[b,n] = exp(-(|c0-ck|^2 + (d0-dk)^2)/2) * |cos(r0, rk)|
    out = relu(gamma * (out + bias - mean)/sqrt(var + eps) + beta)
  with mean/var batch statistics over (B, N) per channel.
  (bias cancels exactly inside train-mode BN, so it is skipped; the center
  tap k=4 has g == 1 exactly -- zero offset and |cos(r,r)| = 1 -- so it is
  fed to the matmul ungated and excluded from the g computation.)

Sharding: data-parallel over B=8 across 8 NeuronCores; per-channel BN
statistics are all-reduced across cores on device.

Per-core plan ([o,n] output orientation, contraction over c on partitions):
 - g for the 8 non-center taps is computed in fp16 on all 128 partitions
   ((tap, group, half)-blocked: 8 taps x 8 groups x 2 halves of 512) in the
   log domain: g = exp(-0.5*((dc+dd) + ln(v*w + 1e-30) - ln(u^2 + 1e-30))).
   The host pre-gathers the shifted/unshifted blocked input views (pure
   layout, no math). g goes to DRAM tile-major (g_lin[group, tap, 1024])
   and is partition-broadcast back [128, 8, 1024] per 2-tile block.
 - per 512-tile: 8 scaled moving tensors t_k = x_shift_k * g_k in bf16 on
   VectorE (each covers both c-chunks via a stride-0 free dim); 36 bf16
   matmuls (9 taps x 2 c-chunks x 2 o-chunks, center tap reads x directly)
   accumulate into PSUM; ScalarE evacuates (Copy + Square) with fused
   per-channel accum_out giving sum / sum-of-squares.
 - stats are all-reduced across the 8 cores; 1/sqrt(var+eps) uses the
   bit-hack + Newton on VectorE (no ACT table switch); BN+ReLU is one
   activation per tile on ScalarE; batched output DMAs.
 - Cross-rep pipelining (the timing harness builds the body R times in one
   NEFF): all tile pools live at build scope with tag-based slot recycling,
   so rep r+1's prologue/main loop overlaps rep r's stats/BN tail purely
   through slot availability. Queues are split so no queue carries both a
   rep's tail traffic and the next rep's prologue: SP carries prologue
   loads + g broadcasts; the gpsimd (Pool) queue carries stats/collective/
   output stores. DMA issue order is pinned with add_dep_helper edges so
   bulk loads cannot jump ahead of the g-critical chain.
"""

import numpy as np
import ml_dtypes

import concourse.bass as bass
import concourse.tile as tile
from concourse import bacc, mybir
from concourse.bass_utils import run_bass_kernel_spmd

NUM_CORES = 8
B, CIN, COUT, N = 8, 256, 256, 8192
K, DIL = 9, 6
PAD = DIL * (K // 2)          # 24
NXP = N + 2 * PAD             # 8240 padded x length
GRP, NGRP = 1024, 8           # g blocked layout: 8 groups of 1024 (= N)
NG = GRP * NGRP               # 8192 blocked g domain (== N)
TAPS = [0, 1, 2, 3, 5, 6, 7, 8]  # center tap k=4 has g == 1 exactly
KG = len(TAPS)
HGRP = 512                    # half-group: g phase runs on 128 partitions
TS, NT = 512, 16              # main loop tiling
BN_EPS = 1e-5
G_EPS = 1e-30
INV_COUNT = 1.0 / (B * N)

F32 = mybir.dt.float32
F16 = mybir.dt.float16
BF16 = mybir.dt.bfloat16
AF = mybir.ActivationFunctionType
ALU = mybir.AluOpType

_CACHE = {}


def _build_kernel(reps=1, single=False, nogate=False, gps_taps=0, use_ar=True,
                  bcast_eng="sync", store_eng="gpsimd", fin_mode="scalar",
                  gb_bufs=2, tm_bufs=9, conv_bufs=19, hoist=0, x_eng="sync",
                  bcast_eng2=None):
    nc = bacc.Bacc(
        "TRN2",
        target_bir_lowering=False,
        debug=False,
        enable_asserts=True,
        num_devices=NUM_CORES,
    )
    xp = nc.dram_tensor("xp", [2, 128, NXP], BF16, kind="ExternalInput").ap()
    cds = nc.dram_tensor("cds", [128, 4, HGRP], F16, kind="ExternalInput").ap()
    cd0 = nc.dram_tensor("cd0", [128, 4, HGRP], F16, kind="ExternalInput").ap()
    ral = nc.dram_tensor("ral", [128, 2, 3, HGRP], F16, kind="ExternalInput").ap()
    wt = nc.dram_tensor("wt", [128, 36, 128], BF16, kind="ExternalInput").ap()
    gam = nc.dram_tensor("gam", [2, 128], F32, kind="ExternalInput").ap()
    bet = nc.dram_tensor("bet", [2, 128], F32, kind="ExternalInput").ap()
    out = nc.dram_tensor("out", [COUT, N], F32, kind="ExternalOutput").ap()

    from contextlib import ExitStack
    with tile.TileContext(nc, pool_alloc_mode="queue") as tc:
        with ExitStack() as ctx:
            e = ctx.enter_context
            P = dict(
                gp=e(tc.tile_pool(name="gp", bufs=1)),
                xw=e(tc.tile_pool(name="xw", bufs=2)),
                conv=e(tc.tile_pool(name="conv", bufs=conv_bufs)),
                gb=e(tc.tile_pool(name="gb", bufs=gb_bufs)),
                tm=e(tc.tile_pool(name="tm", bufs=tm_bufs)),
                ps=e(tc.tile_pool(name="ps", bufs=8, space="PSUM")),
                sq=e(tc.tile_pool(name="sq", bufs=2)),
                fin=e(tc.tile_pool(name="fin", bufs=2)),
                small=e(tc.tile_pool(name="small", bufs=2)),
                const=e(tc.tile_pool(name="const", bufs=1)),
                dram=e(tc.tile_pool(name="dram", bufs=2, space="DRAM")),
            )
            geps = P["const"].tile([128, 1], F32)
            nc.vector.memset(geps, G_EPS)
            for r in range(reps):
                _body(tc, P, geps, xp, cds, cd0, ral, wt, gam, bet, out,
                      single=single, nogate=nogate, gps_taps=gps_taps,
                      use_ar=use_ar, bcast_eng=bcast_eng, store_eng=store_eng,
                      fin_mode=fin_mode, hoist=hoist, x_eng=x_eng,
                      bcast_eng2=bcast_eng2)
    nc.compile()
    return nc


def _body(tc, P, geps, xp, cds, cd0, ral, wt, gam, bet, out, single=False,
          nogate=False, gps_taps=0, use_ar=True, bcast_eng="sync",
          store_eng="gpsimd", fin_mode="scalar", hoist=0, x_eng="sync",
          bcast_eng2=None):
    nc = tc.nc
    from contextlib import nullcontext
    from concourse.tile_rust import add_dep_helper
    hp = (lambda: tc.high_priority(offset=hoist)) if hoist else nullcontext

    g_lin = P["dram"].tile([NGRP, KG, GRP], BF16, tag="glin")

    # ---- phase 1: tap gate g, (tap,group,half)-blocked on 128 partitions ----
    with hp():
        cd_s = P["gp"].tile([128, 4, HGRP], F16, tag="cds")
        cd_0 = P["gp"].tile([128, 4, HGRP], F16, tag="cd0")
        r_all = P["gp"].tile([128, 2, 3, HGRP], F16, tag="ral")
        nc.sync.dma_start(out=cd_s[:], in_=cds[:])
        nc.sync.dma_start(out=cd_0[:], in_=cd0[:])
        i_ral = nc.sync.dma_start(out=r_all[:], in_=ral[:])

        # bulk loads issued after the g-critical inputs
        XLO = 8 * TS + 2 * PAD               # 4144+48 = 4192
        XHI0 = 8 * TS                        # 4096
        w_sb = P["xw"].tile([128, 36, 128], BF16, tag="w")
        i_w = nc.sync.dma_start(out=w_sb[:], in_=wt[:])
        x_lo = P["xw"].tile([128, 2, XLO], BF16, tag="xlo")
        i_xlo = getattr(nc, x_eng).dma_start(out=x_lo[:], in_=bass.AP(
            tensor=xp.tensor, offset=0,
            ap=[[NXP, 128], [128 * NXP, 2], [1, XLO]],
        ))
        add_dep_helper(i_w.ins, i_ral.ins, True, "W load after g-input loads")
        add_dep_helper(i_xlo.ins, i_w.ins, True, "x_lo load after W load")
        gam_sb = P["small"].tile([128, 2], F32, tag="gam")
        nc.sync.dma_start(out=gam_sb[:], in_=bass.AP(
            tensor=gam.tensor, offset=0, ap=[[1, 128], [128, 2]]))
        bet_sb = P["small"].tile([128, 2], F32, tag="bet")
        nc.sync.dma_start(out=bet_sb[:], in_=bass.AP(
            tensor=bet.tensor, offset=0, ap=[[1, 128], [128, 2]]))

        # g chain with in-place reuse (~21KB/partition transient)
        q = P["gp"].tile([128, HGRP], F16, tag="q")
        nc.vector.tensor_sub(cd_s[:], cd_s[:], cd_0[:])    # diff
        nc.vector.tensor_mul(cd_s[:], cd_s[:], cd_s[:])    # diff^2
        nc.vector.tensor_add(q[:], cd_s[:, 0], cd_s[:, 1])
        nc.vector.tensor_add(q[:], q[:], cd_s[:, 2])
        nc.vector.tensor_add(q[:], q[:], cd_s[:, 3])

        rt = P["gp"].tile([128, 3, HGRP], F16, tag="rt")
        nc.vector.tensor_mul(rt[:], r_all[:, 0], r_all[:, 1])
        u = P["gp"].tile([128, HGRP], F16, tag="u")
        nc.vector.tensor_add(u[:], rt[:, 0], rt[:, 1])
        nc.vector.tensor_add(u[:], u[:], rt[:, 2])
        nc.vector.tensor_mul(u[:], u[:], u[:])             # u^2

        nc.vector.tensor_mul(r_all[:], r_all[:], r_all[:])  # r^2
        v = P["gp"].tile([128, HGRP], F16, tag="v")
        nc.vector.tensor_add(v[:], r_all[:, 0, 0], r_all[:, 0, 1])
        nc.vector.tensor_add(v[:], v[:], r_all[:, 0, 2])
        w_ = P["gp"].tile([128, HGRP], F16, tag="w_")
        nc.vector.tensor_add(w_[:], r_all[:, 1, 0], r_all[:, 1, 1])
        nc.vector.tensor_add(w_[:], w_[:], r_all[:, 1, 2])
        nc.vector.tensor_mul(v[:], v[:], w_[:])            # v*w

        nc.scalar.activation(w_[:], v[:], AF.Ln, bias=geps[:])     # ln(vw)
        nc.scalar.activation(rt[:, 0], u[:], AF.Ln, bias=geps[:])  # ln(u^2)
        nc.vector.tensor_add(q[:], q[:], w_[:])
        nc.vector.tensor_sub(q[:], q[:], rt[:, 0])
        gblk = P["gp"].tile([128, HGRP], BF16, tag="gblk")
        nc.scalar.activation(gblk[:], q[:], AF.Exp, scale=-0.5)

        # [128 = (half, tap, group), 512] -> tile-major g_lin[group, tap, :]
        for h in range(2):
            nc.sync.dma_start(out=bass.AP(
                tensor=g_lin.tensor, offset=g_lin.offset + h * TS,
                ap=[[GRP, KG], [KG * GRP, NGRP], [1, TS]],
            ), in_=gblk[h * 64: (h + 1) * 64, :])

        x_hi = P["xw"].tile([128, 2, NXP - XHI0], BF16, tag="xhi")
        i_xhi = getattr(nc, x_eng).dma_start(out=x_hi[:], in_=bass.AP(
            tensor=xp.tensor, offset=8 * TS,
            ap=[[NXP, 128], [128 * NXP, 2], [1, NXP - 8 * TS]],
        ))

    # ---- phase 2: main conv loop over 16 tiles of 512 ----
    s1cols = P["small"].tile([128, 2, NT], F32, tag="s1")
    s2cols = P["small"].tile([128, 2, NT], F32, tag="s2")
    conv_t = []
    gb_t = None
    for t in range(NT):
        xsrc = x_lo if t < 8 else x_hi
        tt = t if t < 8 else t - 8
        if not nogate:
            if t % 2 == 0:
                with (hp() if t == 0 else nullcontext()):
                    gb_t = P["gb"].tile([128, KG, GRP], BF16, tag="gb")
                    _be = (bcast_eng if (bcast_eng2 is None or (t // 2) % 2 == 0)
                           else bcast_eng2)
                    i_gb = getattr(nc, _be).dma_start(
                        out=gb_t[:], in_=bass.AP(
                            tensor=g_lin.tensor,
                            offset=g_lin.offset + (t // 2) * KG * GRP,
                            ap=[[0, 128], [GRP, KG], [1, GRP]],
                        ))
                if t == 0:
                    add_dep_helper(i_xhi.ins, i_gb.ins, True,
                                   "x_hi streams after first g bcast")
            tmul = {}
            half = (t % 2) * TS
            for ki, k in enumerate(TAPS):
                tm = P["tm"].tile([128, 2, TS], BF16, tag="tm")
                gk = gb_t[:, ki, half: half + TS]
                gview = bass.AP(tensor=gk.tensor, offset=gk.offset,
                                ap=[gk.ap[0], [0, 2], [1, TS]])
                xview = xsrc[:, :, k * DIL + tt * TS: k * DIL + tt * TS + TS]
                eng = nc.vector if ki < KG - gps_taps else nc.gpsimd
                eng.tensor_mul(tm[:], xview, gview)
                tmul[k] = tm
        cv = P["conv"].tile([128, 2, TS], BF16, tag="conv")
        conv_t.append(cv)
        for oc in range(2):
            ps = P["ps"].tile([128, TS], F32, tag="ps")
            idx = 0
            for cc in range(2):
                for k in range(K):
                    if nogate or k == 4:
                        rhs = xsrc[:, cc, k * DIL + tt * TS:
                                   k * DIL + tt * TS + TS]
                    else:
                        rhs = tmul[k][:, cc, :]
                    nc.tensor.matmul(
                        ps[:],
                        w_sb[:, k * 4 + cc * 2 + oc, :],
                        rhs,
                        start=(idx == 0),
                        stop=(idx == 17),
                    )
                    idx += 1
            nc.scalar.activation(
                cv[:, oc, :], ps[:], AF.Copy,
                accum_out=s1cols[:, oc, t: t + 1],
            )
            sq = P["sq"].tile([128, TS], BF16, tag="sq")
            nc.scalar.activation(
                sq[:], ps[:], AF.Square,
                accum_out=s2cols[:, oc, t: t + 1],
            )

    # ---- phase 3: stats all-reduce + BN coefficients ----
    stats = P["small"].tile([128, 4], F32, tag="stats")
    nc.vector.tensor_reduce(stats[:, 0:2], s1cols[:], axis=mybir.AxisListType.X,
                            op=ALU.add)
    nc.vector.tensor_reduce(stats[:, 2:4], s2cols[:], axis=mybir.AxisListType.X,
                            op=ALU.add)
    cc_in = P["dram"].tile([128, 4], F32, tag="ccin")
    cc_out = P["dram"].tile([NUM_CORES * 128, 4], F32, tag="ccout")
    nc.gpsimd.dma_start(out=cc_in[:], in_=stats[:])
    red = P["small"].tile([128, 4], F32, tag="red")
    if single:
        nc.gpsimd.dma_start(out=cc_out[0:128, :], in_=cc_in[:])
        nc.gpsimd.dma_start(out=red[:], in_=cc_out[0:128, :])
    elif use_ar:
        nc.gpsimd.collective_compute(
            "AllReduce", ALU.add,
            replica_groups=[list(range(NUM_CORES))],
            ins=[cc_in.opt()], outs=[cc_out[0:128, :].opt()],
        )
        nc.gpsimd.dma_start(out=red[:], in_=cc_out[0:128, :])
    else:
        nc.gpsimd.collective_compute(
            "AllGather", ALU.bypass,
            replica_groups=[list(range(NUM_CORES))],
            ins=[cc_in.opt()], outs=[cc_out.opt()],
        )
        red8 = P["small"].tile([128, 4, NUM_CORES], F32, tag="red8")
        nc.gpsimd.dma_start(out=red8[:], in_=bass.AP(
            tensor=cc_out.tensor, offset=cc_out.offset,
            ap=[[4, 128], [1, 4], [512, NUM_CORES]],
        ))
        nc.vector.tensor_reduce(red[:], red8[:], axis=mybir.AxisListType.X,
                                op=ALU.add)

    me2 = P["small"].tile([128, 4], F32, tag="me2")
    nc.vector.tensor_scalar_mul(me2[:], red[:], INV_COUNT)
    m = me2[:, 0:2]
    var = P["small"].tile([128, 2], F32, tag="var")
    nc.vector.tensor_mul(var[:], m, m)
    nc.vector.tensor_sub(var[:], me2[:, 2:4], var[:])
    nc.vector.tensor_scalar_add(var[:], var[:], BN_EPS)
    # rinv = rsqrt(var+eps): bit-hack + Newton on DVE (no ACT table switch)
    vi = var[:].bitcast(mybir.dt.int32)
    yi = P["small"].tile([128, 2], mybir.dt.int32, tag="yi")
    nc.vector.tensor_scalar(out=yi[:], in0=vi[:], scalar1=1, scalar2=None,
                            op0=ALU.arith_shift_right)
    nc.vector.tensor_scalar(out=yi[:], in0=yi[:], scalar1=-1,
                            scalar2=0x5F3759DF, op0=ALU.mult, op1=ALU.add)
    rinv_t = P["small"].tile([128, 2], F32, tag="rinv")
    rinv = rinv_t[:]
    nc.vector.tensor_copy(rinv, yi[:].bitcast(F32))
    hv = P["small"].tile([128, 2], F32, tag="hv")
    nc.vector.tensor_scalar_mul(hv[:], var[:], -0.5)
    yy = P["small"].tile([128, 2], F32, tag="yy")
    for _ in range(2):
        nc.vector.tensor_mul(yy[:], rinv, rinv)
        nc.vector.tensor_mul(yy[:], yy[:], hv[:])
        nc.vector.tensor_scalar_add(yy[:], yy[:], 1.5)
        nc.vector.tensor_mul(rinv, rinv, yy[:])
    scl = P["small"].tile([128, 2], F32, tag="scl")
    nc.vector.tensor_mul(scl[:], rinv, gam_sb[:])
    bia = P["small"].tile([128, 2], F32, tag="bia")
    nc.vector.tensor_mul(bia[:], m, scl[:])
    nc.vector.tensor_sub(bia[:], bet_sb[:], bia[:])

    # ---- phase 4: BN + ReLU + store (batched DMA on the tail queue) ----
    QT = 2  # tiles per output DMA
    for tq in range(NT // QT):
        for oc in range(2):
            fin = P["fin"].tile([128, QT, TS], F32, tag="fin")
            for j in range(QT):
                t = tq * QT + j
                use_scalar = (fin_mode == "scalar"
                              or (fin_mode == "split"
                                  and (2 * t + oc + j) % 2 == 0))
                if use_scalar:
                    nc.scalar.activation(
                        fin[:, j, :], conv_t[t][:, oc, :], AF.Relu,
                        bias=bia[:, oc: oc + 1], scale=scl[:, oc: oc + 1],
                    )
                else:
                    nc.vector.tensor_scalar(
                        out=fin[:, j, :], in0=conv_t[t][:, oc, :],
                        scalar1=scl[:, oc: oc + 1], scalar2=bia[:, oc: oc + 1],
                        op0=ALU.mult, op1=ALU.add,
                    )
                    nc.vector.tensor_scalar_max(fin[:, j, :], fin[:, j, :], 0.0)
            getattr(nc, store_eng).dma_start(
                out=out[oc * 128: (oc + 1) * 128,
                        tq * QT * TS: (tq + 1) * QT * TS],
                in_=fin[:].rearrange("p q t -> p (q t)"),
            )


def _prep_inputs(x, coords, rotations, distances, W, gamma, beta):
    """Host-side sharding/layout prep. Returns per-core input maps."""
    bf = ml_dtypes.bfloat16
    # weights: [o, c, k] -> 36 lhsT tiles [(k, cc, oc), c, o]
    wt = W.reshape(2, 128, 2, 128, K)            # [oc, o, cc, c, k]
    wt = wt.transpose(3, 4, 2, 0, 1)             # [c, k, cc, oc, o]
    wt = np.ascontiguousarray(wt.reshape(128, 36, 128), dtype=bf)
    gam2 = np.ascontiguousarray(gamma.reshape(2, 128), dtype=np.float32)
    bet2 = np.ascontiguousarray(beta.reshape(2, 128), dtype=np.float32)

    # gather index for the (half, tap, group)-blocked g layout
    ks = np.array(TAPS)
    idx = ((np.arange(2) * HGRP)[:, None, None, None]
           + (ks * DIL)[None, :, None, None]
           + (np.arange(NGRP) * GRP)[None, None, :, None]
           + np.arange(HGRP)[None, None, None, :])    # [2, KG, NGRP, HGRP]
    idx0 = idx - (ks * DIL)[None, :, None, None] + PAD
    in_maps = []
    for b in range(NUM_CORES):
        xpad = np.zeros((CIN, NXP), dtype=bf)
        xpad[:, PAD: PAD + N] = x[b].astype(bf)
        cd4 = np.zeros((4, NXP), dtype=np.float32)
        cd4[:3, PAD: PAD + N] = coords[b]
        cd4[3, PAD: PAD + N] = distances[b]
        rot = np.zeros((3, NXP), dtype=np.float32)
        rot[:, PAD: PAD + N] = rotations[b]
        cds_h = cd4[:, idx].transpose(1, 2, 3, 0, 4).reshape(128, 4, HGRP)
        cd0_h = cd4[:, idx0].transpose(1, 2, 3, 0, 4).reshape(128, 4, HGRP)
        r_s = rot[:, idx].transpose(1, 2, 3, 0, 4)    # [2, KG, NGRP, 3, HGRP]
        r_0 = rot[:, idx0].transpose(1, 2, 3, 0, 4)
        ral_h = np.stack([r_0, r_s], axis=3).reshape(128, 2, 3, HGRP)
        in_maps.append({
            "xp": np.ascontiguousarray(xpad.reshape(2, 128, NXP)),
            "cds": np.ascontiguousarray(cds_h.astype(np.float16)),
            "cd0": np.ascontiguousarray(cd0_h.astype(np.float16)),
            "ral": np.ascontiguousarray(ral_h.astype(np.float16)),
            "wt": wt,
            "gam": gam2,
            "bet": bet2,
        })
    return in_maps


def kernel(x, coords, rotations, distances, W, bias, gamma, beta):
    # accept jax arrays / array-likes as produced by reference.setup_inputs()
    x, coords, rotations, distances, W, gamma, beta = (
        np.asarray(a, dtype=np.float32)
        for a in (x, coords, rotations, distances, W, gamma, beta))
    if "nc" not in _CACHE:
        _CACHE["nc"] = _build_kernel()
    nc = _CACHE["nc"]
    in_maps = _prep_inputs(x, coords, rotations, distances, W, gamma, beta)
    res = run_bass_kernel_spmd(nc, in_maps, list(range(NUM_CORES)), trace=False)
    return np.stack([res.results[b]["out"] for b in range(NUM_CORES)], axis=0)



# revision 23
# speedup vs baseline: 1.6653x; 1.6653x over previous
"""Trainium2 Bass kernel for nn_ASPPConv (gated dilated conv + BatchNorm + ReLU).

Reference computation (per batch element b):
    for k in 0..9:  out[b] += W[:,:,k] @ (x_shift_k[b] * g_k[b])
    g_k[b,n] = exp(-(|c0-ck|^2 + (d0-dk)^2)/2) * |cos(r0, rk)|
    out = relu(gamma * (out + bias - mean)/sqrt(var + eps) + beta)
  with mean/var batch statistics over (B, N) per channel.
  (bias cancels exactly inside train-mode BN, so it is skipped; the center
  tap k=4 has g == 1 exactly -- zero offset and |cos(r,r)| = 1 -- so it is
  fed to the matmul ungated and excluded from the g computation.)

Sharding: data-parallel over B=8 across 8 NeuronCores; per-channel BN
statistics are all-reduced across cores on device.

Per-core plan ([o,n] output orientation, contraction over c on partitions):
 - g for the 8 non-center taps is computed in fp16 on all 128 partitions
   ((tap, group, half)-blocked: 8 taps x 8 groups x 2 halves of 512) in the
   log domain: g = exp(-0.5*((dc+dd) + ln(v*w + 1e-30) - ln(u^2 + 1e-30))).
   The host pre-gathers the shifted/unshifted blocked input views (pure
   layout, no math). g goes to DRAM tile-major (g_lin[group, tap, 1024])
   and is partition-broadcast back [128, 8, 1024] per 2-tile block.
 - per 512-tile: 8 scaled moving tensors t_k = x_shift_k * g_k in bf16 on
   VectorE (each covers both c-chunks via a stride-0 free dim); 36 bf16
   matmuls (9 taps x 2 c-chunks x 2 o-chunks, center tap reads x directly)
   accumulate into PSUM; ScalarE evacuates (Copy + Square) with fused
   per-channel accum_out giving sum / sum-of-squares.
 - stats are all-reduced across the 8 cores; 1/sqrt(var+eps) uses the
   bit-hack + Newton on VectorE (no ACT table switch); BN+ReLU is one
   activation per tile on ScalarE; batched output DMAs.
 - Cross-rep pipelining (the timing harness builds the body R times in one
   NEFF): all tile pools live at build scope with tag-based slot recycling,
   so rep r+1's prologue/main loop overlaps rep r's stats/BN tail purely
   through slot availability. Queues are split so no queue carries both a
   rep's tail traffic and the next rep's prologue: SP carries prologue
   loads + g broadcasts; the gpsimd (Pool) queue carries stats/collective/
   output stores. DMA issue order is pinned with add_dep_helper edges so
   bulk loads cannot jump ahead of the g-critical chain.
"""

import numpy as np
import ml_dtypes

import concourse.bass as bass
import concourse.tile as tile
from concourse import bacc, mybir
from concourse.bass_utils import run_bass_kernel_spmd

NUM_CORES = 8
B, CIN, COUT, N = 8, 256, 256, 8192
K, DIL = 9, 6
PAD = DIL * (K // 2)          # 24
NXP = N + 2 * PAD             # 8240 padded x length
GRP, NGRP = 1024, 8           # g blocked layout: 8 groups of 1024 (= N)
NG = GRP * NGRP               # 8192 blocked g domain (== N)
TAPS = [0, 1, 2, 3, 5, 6, 7, 8]  # center tap k=4 has g == 1 exactly
KG = len(TAPS)
HGRP = 512                    # half-group: g phase runs on 128 partitions
TS, NT = 512, 16              # main loop tiling
BN_EPS = 1e-5
G_EPS = 1e-30
INV_COUNT = 1.0 / (B * N)

F32 = mybir.dt.float32
F16 = mybir.dt.float16
BF16 = mybir.dt.bfloat16
AF = mybir.ActivationFunctionType
ALU = mybir.AluOpType

_CACHE = {}


def _build_kernel(reps=1, single=False, nogate=False, gps_taps=0, use_ar=True,
                  bcast_eng="sync", store_eng="gpsimd", fin_mode="scalar",
                  gb_bufs=2, tm_bufs=9, conv_bufs=19, hoist=0, x_eng="sync",
                  bcast_eng2=None, g_eng="vector", korder="std", gb_split=1,
                  coef_eng="vector", hsegs=(900, 550, 430, 310), gb_mode="2tile",
                  nobc=False, fin_bufs=2, qt=2, pair_tm=False):
    nc = bacc.Bacc(
        "TRN2",
        target_bir_lowering=False,
        debug=False,
        enable_asserts=True,
        num_devices=NUM_CORES,
    )
    xp = nc.dram_tensor("xp", [2, 128, NXP], BF16, kind="ExternalInput").ap()
    cds = nc.dram_tensor("cds", [128, 4, HGRP], F16, kind="ExternalInput").ap()
    cd0 = nc.dram_tensor("cd0", [128, 4, HGRP], F16, kind="ExternalInput").ap()
    ral = nc.dram_tensor("ral", [128, 2, 3, HGRP], F16, kind="ExternalInput").ap()
    wt = nc.dram_tensor("wt", [128, 36, 128], BF16, kind="ExternalInput").ap()
    gam = nc.dram_tensor("gam", [2, 128], F32, kind="ExternalInput").ap()
    bet = nc.dram_tensor("bet", [2, 128], F32, kind="ExternalInput").ap()
    out = nc.dram_tensor("out", [COUT, N], F32, kind="ExternalOutput").ap()

    from contextlib import ExitStack
    with tile.TileContext(nc, pool_alloc_mode="queue") as tc:
        with ExitStack() as ctx:
            e = ctx.enter_context
            P = dict(
                gp=e(tc.tile_pool(name="gp", bufs=1)),
                gpin=e(tc.tile_pool(name="gpin", bufs=2)),
                xw=e(tc.tile_pool(name="xw", bufs=2)),
                xp1=e(tc.tile_pool(name="xp1", bufs=1)),
                wp=e(tc.tile_pool(name="wp", bufs=2)),
                conv=e(tc.tile_pool(name="conv", bufs=conv_bufs)),
                gb=e(tc.tile_pool(name="gb", bufs=gb_bufs)),
                tm=e(tc.tile_pool(name="tm", bufs=tm_bufs)),
                ps=e(tc.tile_pool(name="ps", bufs=8, space="PSUM")),
                sq=e(tc.tile_pool(name="sq", bufs=1)),
                fin=e(tc.tile_pool(name="fin", bufs=fin_bufs)),
                small=e(tc.tile_pool(name="small", bufs=2)),
                const=e(tc.tile_pool(name="const", bufs=1)),
                dram=e(tc.tile_pool(name="dram", bufs=2, space="DRAM")),
            )
            geps = P["const"].tile([128, 1], F32)
            nc.vector.memset(geps, G_EPS)
            # int/float consts for Pool-engine bit-hack chains (TensorScalar
            # is not available on Pool, so constants come from SBUF views)
            cst = P["const"].tile([128, 5], mybir.dt.int32)
            nc.vector.memset(cst[:, 0:1], 0x7FFFFFFF)
            nc.vector.memset(cst[:, 1:2], 1)
            nc.vector.memset(cst[:, 2:3], 0x5F3759DF)
            nc.vector.memset(cst[:, 3:4].bitcast(F32), -0.5)
            nc.vector.memset(cst[:, 4:5].bitcast(F32), 1.5)
            for r in range(reps):
                _body(tc, P, geps, cst, xp, cds, cd0, ral, wt, gam, bet, out,
                      single=single, nogate=nogate, gps_taps=gps_taps,
                      use_ar=use_ar, bcast_eng=bcast_eng, store_eng=store_eng,
                      fin_mode=fin_mode, hoist=hoist, x_eng=x_eng,
                      bcast_eng2=bcast_eng2, g_eng=g_eng, korder=korder,
                      gb_split=gb_split, coef_eng=coef_eng, hsegs=hsegs,
                      gb_mode=gb_mode, nobc=nobc, qt=qt, pair_tm=pair_tm)
    nc.compile()
    return nc


def _body(tc, P, geps, cst, xp, cds, cd0, ral, wt, gam, bet, out, single=False,
          nogate=False, gps_taps=0, use_ar=True, bcast_eng="sync",
          store_eng="gpsimd", fin_mode="scalar", hoist=0, x_eng="sync",
          bcast_eng2=None, g_eng="vector", korder="std", gb_split=1,
          coef_eng="vector", hsegs=(900, 550, 430, 310), gb_mode="2tile",
          nobc=False, qt=2, pair_tm=False):
    nc = tc.nc
    from contextlib import nullcontext
    from concourse.tile_rust import add_dep_helper
    hp = (lambda: tc.high_priority(offset=hoist)) if hoist else nullcontext

    g_lin = P["dram"].tile([NGRP, KG, GRP], BF16, tag="glin")
    XLO = 8 * TS + 2 * PAD               # 4144+48 = 4192
    XHI0 = 8 * TS                        # 4096

    # ---- phase 1: tap gate g, (tap,group,half)-blocked on 128 partitions ----
    if g_eng == "v3":
        # DVE chain in graduated hoisted segments: each segment lands in rep
        # r's DVE inter-tile idle gaps (DVE is tm-slot-capped ~2.8us/tile
        # idle), so rep r+1's g is ready before rep r's matmuls end.  Inputs
        # are double-buffered (gpin) so the loads prefetch mid-rep; W/x_lo
        # ride the scalar HWDGE ring to stay clear of the SP gb traffic.
        h1, h2, h3, h4 = hsegs
        V = nc.vector
        with tc.high_priority(offset=h1):
            cd_s = P["gpin"].tile([128, 4, HGRP], F16, tag="cds")
            cd_0 = P["gpin"].tile([128, 4, HGRP], F16, tag="cd0")
            r_all = P["gpin"].tile([128, 2, 3, HGRP], F16, tag="ral")
            nc.sync.dma_start(out=cd_s[:], in_=cds[:])
            nc.sync.dma_start(out=cd_0[:], in_=cd0[:])
            nc.sync.dma_start(out=r_all[:], in_=ral[:])
            w_sb = P["wp"].tile([128, 36, 128], BF16, tag="w")
            nc.scalar.dma_start(out=w_sb[:], in_=wt[:])
            x_lo = P["xp1"].tile([128, 2, XLO], BF16, tag="xlo")
            nc.scalar.dma_start(out=x_lo[:], in_=bass.AP(
                tensor=xp.tensor, offset=0,
                ap=[[NXP, 128], [128 * NXP, 2], [1, XLO]],
            ))
            gam_sb = P["small"].tile([128, 2], F32, tag="gam")
            nc.scalar.dma_start(out=gam_sb[:], in_=bass.AP(
                tensor=gam.tensor, offset=0, ap=[[1, 128], [128, 2]]))
            bet_sb = P["small"].tile([128, 2], F32, tag="bet")
            nc.scalar.dma_start(out=bet_sb[:], in_=bass.AP(
                tensor=bet.tensor, offset=0, ap=[[1, 128], [128, 2]]))
        sc = P["gp"].tile([128, 2, HGRP], F16, tag="sc")
        rt = P["gp"].tile([128, 3, HGRP], F16, tag="rt")
        q, u = sc[:, 0], sc[:, 1]
        with tc.high_priority(offset=h2):
            V.tensor_sub(cd_s[:], cd_s[:], cd_0[:])       # diff
            V.tensor_mul(cd_s[:], cd_s[:], cd_s[:])       # diff^2
            V.tensor_add(q, cd_s[:, 0], cd_s[:, 1])
            V.tensor_add(q, q, cd_s[:, 2])
            V.tensor_add(q, q, cd_s[:, 3])                # dc+dd
        with tc.high_priority(offset=h3):
            V.tensor_mul(rt[:], r_all[:, 0], r_all[:, 1])
            V.tensor_add(u, rt[:, 0], rt[:, 1])
            V.tensor_add(u, u, rt[:, 2])
            V.tensor_mul(u, u, u)                         # u^2
            V.tensor_mul(r_all[:], r_all[:], r_all[:])    # r^2
            V.tensor_add(rt[:, 0], r_all[:, 0, 0], r_all[:, 0, 1])
            V.tensor_add(rt[:, 0], rt[:, 0], r_all[:, 0, 2])   # v
            V.tensor_add(rt[:, 1], r_all[:, 1, 0], r_all[:, 1, 1])
            V.tensor_add(rt[:, 1], rt[:, 1], r_all[:, 1, 2])   # w
            V.tensor_mul(rt[:, 0], rt[:, 0], rt[:, 1])    # v*w
        gblk = P["gp"].tile([128, HGRP], BF16, tag="gblk")
        with tc.high_priority(offset=h4):
            nc.scalar.activation(rt[:, 2], rt[:, 0], AF.Ln, bias=geps[:])
            nc.scalar.activation(rt[:, 1], u, AF.Ln, bias=geps[:])  # ln(u^2)
            V.tensor_add(q, q, rt[:, 2])
            V.tensor_sub(q, q, rt[:, 1])
            nc.scalar.activation(gblk[:], q, AF.Exp, scale=-0.5)
        # natural priority: g stores then first gb ride SP right behind rep
        # r's last broadcasts; x_hi follows (its WAR clears at rep r's end)
        for h in range(2):
            nc.sync.dma_start(out=bass.AP(
                tensor=g_lin.tensor, offset=g_lin.offset + h * TS,
                ap=[[GRP, KG], [KG * GRP, NGRP], [1, TS]],
            ), in_=gblk[h * 64: (h + 1) * 64, :])
        x_hi = P["xp1"].tile([128, 2, NXP - XHI0], BF16, tag="xhi")
        i_xhi = nc.sync.dma_start(out=x_hi[:], in_=bass.AP(
            tensor=xp.tensor, offset=8 * TS,
            ap=[[NXP, 128], [128 * NXP, 2], [1, NXP - 8 * TS]],
        ))
    elif g_eng == "pool":
        # g chain on the (otherwise idle mid-rep) gpsimd engine so rep r+1's
        # chain overlaps rep r's main loop.  ScalarE only runs the 3 small
        # Ln/Ln/Exp activations (same table), hoisted into rep r's stream.
        E = nc.gpsimd
        with hp():
            cd_s = P["gp"].tile([128, 4, HGRP], F16, tag="cds")
            s16 = P["gp"].tile([128, 7, HGRP], F16, tag="s16")
            cd_0 = s16[:, 0:4]
            r_all = P["gp"].tile([128, 2, 3, HGRP], F16, tag="ral")
            nc.sync.dma_start(out=cd_s[:], in_=cds[:])
            nc.sync.dma_start(out=cd_0, in_=cd0[:])
            i_ral = nc.sync.dma_start(out=r_all[:], in_=ral[:])

            E.tensor_sub(cd_s[:], cd_s[:], cd_0)           # diff
            E.tensor_mul(cd_s[:], cd_s[:], cd_s[:])        # diff^2
            q = s16[:, 0]
            E.tensor_add(q, cd_s[:, 0], cd_s[:, 1])
            E.tensor_add(q, q, cd_s[:, 2])
            E.tensor_add(q, q, cd_s[:, 3])                 # dc+dd

            rt = s16[:, 4:7]
            E.tensor_mul(rt, r_all[:, 0], r_all[:, 1])
            u2 = s16[:, 1]
            E.tensor_add(u2, rt[:, 0], rt[:, 1])
            E.tensor_add(u2, u2, rt[:, 2])
            E.tensor_mul(u2, u2, u2)                       # u^2
            E.tensor_mul(r_all[:], r_all[:], r_all[:])     # r^2
            vw = s16[:, 2]
            E.tensor_add(vw, r_all[:, 0, 0], r_all[:, 0, 1])
            E.tensor_add(vw, vw, r_all[:, 0, 2])           # v
            w2 = s16[:, 3]
            E.tensor_add(w2, r_all[:, 1, 0], r_all[:, 1, 1])
            E.tensor_add(w2, w2, r_all[:, 1, 2])           # w
            E.tensor_mul(vw, vw, w2)                       # v*w

            lnvw = s16[:, 3]                               # w2 is dead
            nc.scalar.activation(lnvw, vw, AF.Ln, bias=geps[:])
            lnu2 = s16[:, 4]                               # rt row 0 is dead
            nc.scalar.activation(lnu2, u2, AF.Ln, bias=geps[:])
            E.tensor_add(q, q, lnvw)
            E.tensor_sub(q, q, lnu2)
            gblk = P["gp"].tile([128, HGRP], BF16, tag="gblk")
            nc.scalar.activation(gblk[:], q, AF.Exp, scale=-0.5)

        # bulk loads and g stores at natural (unhoisted) priority: they sit
        # after rep r's SP traffic and drain during r's tail
        w_sb = P["xw"].tile([128, 36, 128], BF16, tag="w")
        i_w = nc.sync.dma_start(out=w_sb[:], in_=wt[:])
        x_lo = P["xw"].tile([128, 2, XLO], BF16, tag="xlo")
        i_xlo = getattr(nc, x_eng).dma_start(out=x_lo[:], in_=bass.AP(
            tensor=xp.tensor, offset=0,
            ap=[[NXP, 128], [128 * NXP, 2], [1, XLO]],
        ))
        add_dep_helper(i_xlo.ins, i_w.ins, True, "x_lo load after W load")
        gam_sb = P["small"].tile([128, 2], F32, tag="gam")
        nc.sync.dma_start(out=gam_sb[:], in_=bass.AP(
            tensor=gam.tensor, offset=0, ap=[[1, 128], [128, 2]]))
        bet_sb = P["small"].tile([128, 2], F32, tag="bet")
        nc.sync.dma_start(out=bet_sb[:], in_=bass.AP(
            tensor=bet.tensor, offset=0, ap=[[1, 128], [128, 2]]))
        for h in range(2):
            nc.sync.dma_start(out=bass.AP(
                tensor=g_lin.tensor, offset=g_lin.offset + h * TS,
                ap=[[GRP, KG], [KG * GRP, NGRP], [1, TS]],
            ), in_=gblk[h * 64: (h + 1) * 64, :])
        x_hi = P["xw"].tile([128, 2, NXP - XHI0], BF16, tag="xhi")
        i_xhi = getattr(nc, x_eng).dma_start(out=x_hi[:], in_=bass.AP(
            tensor=xp.tensor, offset=8 * TS,
            ap=[[NXP, 128], [128 * NXP, 2], [1, NXP - 8 * TS]],
        ))
    else:
      with hp():
        cd_s = P["gp"].tile([128, 4, HGRP], F16, tag="cds")
        cd_0 = P["gp"].tile([128, 4, HGRP], F16, tag="cd0")
        r_all = P["gp"].tile([128, 2, 3, HGRP], F16, tag="ral")
        nc.sync.dma_start(out=cd_s[:], in_=cds[:])
        nc.sync.dma_start(out=cd_0[:], in_=cd0[:])
        i_ral = nc.sync.dma_start(out=r_all[:], in_=ral[:])

        # bulk loads issued after the g-critical inputs
        w_sb = P["xw"].tile([128, 36, 128], BF16, tag="w")
        i_w = nc.sync.dma_start(out=w_sb[:], in_=wt[:])
        x_lo = P["xw"].tile([128, 2, XLO], BF16, tag="xlo")
        i_xlo = getattr(nc, x_eng).dma_start(out=x_lo[:], in_=bass.AP(
            tensor=xp.tensor, offset=0,
            ap=[[NXP, 128], [128 * NXP, 2], [1, XLO]],
        ))
        add_dep_helper(i_w.ins, i_ral.ins, True, "W load after g-input loads")
        add_dep_helper(i_xlo.ins, i_w.ins, True, "x_lo load after W load")
        gam_sb = P["small"].tile([128, 2], F32, tag="gam")
        nc.sync.dma_start(out=gam_sb[:], in_=bass.AP(
            tensor=gam.tensor, offset=0, ap=[[1, 128], [128, 2]]))
        bet_sb = P["small"].tile([128, 2], F32, tag="bet")
        nc.sync.dma_start(out=bet_sb[:], in_=bass.AP(
            tensor=bet.tensor, offset=0, ap=[[1, 128], [128, 2]]))

        # g chain with in-place reuse (~21KB/partition transient)
        q = P["gp"].tile([128, HGRP], F16, tag="q")
        nc.vector.tensor_sub(cd_s[:], cd_s[:], cd_0[:])    # diff
        nc.vector.tensor_mul(cd_s[:], cd_s[:], cd_s[:])    # diff^2
        nc.vector.tensor_add(q[:], cd_s[:, 0], cd_s[:, 1])
        nc.vector.tensor_add(q[:], q[:], cd_s[:, 2])
        nc.vector.tensor_add(q[:], q[:], cd_s[:, 3])

        rt = P["gp"].tile([128, 3, HGRP], F16, tag="rt")
        nc.vector.tensor_mul(rt[:], r_all[:, 0], r_all[:, 1])
        u = P["gp"].tile([128, HGRP], F16, tag="u")
        nc.vector.tensor_add(u[:], rt[:, 0], rt[:, 1])
        nc.vector.tensor_add(u[:], u[:], rt[:, 2])
        nc.vector.tensor_mul(u[:], u[:], u[:])             # u^2

        nc.vector.tensor_mul(r_all[:], r_all[:], r_all[:])  # r^2
        v = P["gp"].tile([128, HGRP], F16, tag="v")
        nc.vector.tensor_add(v[:], r_all[:, 0, 0], r_all[:, 0, 1])
        nc.vector.tensor_add(v[:], v[:], r_all[:, 0, 2])
        w_ = P["gp"].tile([128, HGRP], F16, tag="w_")
        nc.vector.tensor_add(w_[:], r_all[:, 1, 0], r_all[:, 1, 1])
        nc.vector.tensor_add(w_[:], w_[:], r_all[:, 1, 2])
        nc.vector.tensor_mul(v[:], v[:], w_[:])            # v*w

        nc.scalar.activation(w_[:], v[:], AF.Ln, bias=geps[:])     # ln(vw)
        nc.scalar.activation(rt[:, 0], u[:], AF.Ln, bias=geps[:])  # ln(u^2)
        nc.vector.tensor_add(q[:], q[:], w_[:])
        nc.vector.tensor_sub(q[:], q[:], rt[:, 0])
        gblk = P["gp"].tile([128, HGRP], BF16, tag="gblk")
        nc.scalar.activation(gblk[:], q[:], AF.Exp, scale=-0.5)

        # [128 = (half, tap, group), 512] -> tile-major g_lin[group, tap, :]
        for h in range(2):
            nc.sync.dma_start(out=bass.AP(
                tensor=g_lin.tensor, offset=g_lin.offset + h * TS,
                ap=[[GRP, KG], [KG * GRP, NGRP], [1, TS]],
            ), in_=gblk[h * 64: (h + 1) * 64, :])

        x_hi = P["xw"].tile([128, 2, NXP - XHI0], BF16, tag="xhi")
        i_xhi = getattr(nc, x_eng).dma_start(out=x_hi[:], in_=bass.AP(
            tensor=xp.tensor, offset=8 * TS,
            ap=[[NXP, 128], [128 * NXP, 2], [1, NXP - 8 * TS]],
        ))

    # ---- phase 2: main conv loop over 16 tiles of 512 ----
    s1cols = P["small"].tile([128, 2, NT], F32, tag="s1")
    s2cols = P["small"].tile([128, 2, NT], F32, tag="s2")
    conv_t = []
    gb_t = None
    for t in range(NT):
        xsrc = x_lo if t < 8 else x_hi
        tt = t if t < 8 else t - 8
        if not nogate:
            if gb_mode == "1tile":
                gb_t = P["gb"].tile([128, KG, TS], BF16, tag="gb")
                _be = (bcast_eng if (bcast_eng2 is None or t % 2 == 0)
                       else bcast_eng2)
                if not nobc:
                    i_gb = getattr(nc, _be).dma_start(
                        out=gb_t[:], in_=bass.AP(
                            tensor=g_lin.tensor,
                            offset=(g_lin.offset + (t // 2) * KG * GRP
                                    + (t % 2) * TS),
                            ap=[[0, 128], [GRP, KG], [1, TS]],
                        ))
                    if t == 0:
                        add_dep_helper(i_xhi.ins, i_gb.ins, True,
                                       "x_hi streams after first g bcast")
                tmul = {}
                for ki, k in enumerate(TAPS):
                    tm = P["tm"].tile([128, 2, TS], BF16, tag="tm")
                    gk = gb_t[:, ki, :]
                    gview = bass.AP(tensor=gk.tensor, offset=gk.offset,
                                    ap=[gk.ap[0], [0, 2], [1, TS]])
                    xview = xsrc[:, :, k * DIL + tt * TS: k * DIL + tt * TS + TS]
                    eng = nc.vector if ki < KG - gps_taps else nc.gpsimd
                    eng.tensor_mul(tm[:], xview, gview)
                    tmul[k] = tm
            elif t % 2 == 0 and nobc:
                # timing probe: single broadcast per rep, reused (wrong g)
                if t == 0:
                    gb_t = P["gb"].tile([128, KG, GRP], BF16, tag="gb")
                    i_gb = getattr(nc, bcast_eng).dma_start(
                        out=gb_t[:], in_=bass.AP(
                            tensor=g_lin.tensor, offset=g_lin.offset,
                            ap=[[0, 128], [GRP, KG], [1, GRP]],
                        ))
                    add_dep_helper(i_xhi.ins, i_gb.ins, True,
                                   "x_hi streams after first g bcast")
            elif t % 2 == 0:
                with (hp() if (t == 0 and g_eng == "vector") else nullcontext()):
                    gb_t = P["gb"].tile([128, KG, GRP], BF16, tag="gb")
                    _be = (bcast_eng if (bcast_eng2 is None or (t // 2) % 2 == 0)
                           else bcast_eng2)
                    if t == 0 and gb_split == 2:
                        for hh in range(2):
                            i_gb = getattr(nc, _be).dma_start(
                                out=gb_t[:, hh * (KG // 2): (hh + 1) * (KG // 2), :],
                                in_=bass.AP(
                                    tensor=g_lin.tensor,
                                    offset=(g_lin.offset + (t // 2) * KG * GRP
                                            + hh * (KG // 2) * GRP),
                                    ap=[[0, 128], [GRP, KG // 2], [1, GRP]],
                                ))
                    else:
                        i_gb = getattr(nc, _be).dma_start(
                            out=gb_t[:], in_=bass.AP(
                                tensor=g_lin.tensor,
                                offset=g_lin.offset + (t // 2) * KG * GRP,
                                ap=[[0, 128], [GRP, KG], [1, GRP]],
                            ))
                if t == 0:
                    add_dep_helper(i_xhi.ins, i_gb.ins, True,
                                   "x_hi streams after first g bcast")
            if gb_mode != "1tile" and pair_tm:
                # one DVE mul per tap PAIR: halves per-op dispatch overhead
                tmul = {}
                half = (t % 2) * TS
                for pi in range(4):
                    k1, k2 = TAPS[2 * pi], TAPS[2 * pi + 1]
                    tm2 = P["tm"].tile([128, 2, 2, TS], BF16, tag="tm")
                    gk = gb_t[:, 2 * pi, half: half + TS]
                    gview = bass.AP(tensor=gk.tensor, offset=gk.offset,
                                    ap=[gk.ap[0], [GRP, 2], [0, 2], [1, TS]])
                    xb = xsrc[:, :, k1 * DIL + tt * TS: k1 * DIL + tt * TS + TS]
                    xview = bass.AP(tensor=xb.tensor, offset=xb.offset,
                                    ap=[xb.ap[0], [DIL * (k2 - k1), 2],
                                        xb.ap[1], xb.ap[2]])
                    nc.vector.tensor_mul(tm2[:], xview, gview)
                    tmul[k1] = (tm2, 0)
                    tmul[k2] = (tm2, 1)
            elif gb_mode != "1tile":
                tmul = {}
                half = (t % 2) * TS
                for ki, k in enumerate(TAPS):
                    tm = P["tm"].tile([128, 2, TS], BF16, tag="tm")
                    gk = gb_t[:, ki, half: half + TS]
                    gview = bass.AP(tensor=gk.tensor, offset=gk.offset,
                                    ap=[gk.ap[0], [0, 2], [1, TS]])
                    xview = xsrc[:, :, k * DIL + tt * TS: k * DIL + tt * TS + TS]
                    eng = nc.vector if ki < KG - gps_taps else nc.gpsimd
                    eng.tensor_mul(tm[:], xview, gview)
                    tmul[k] = tm
        cv = P["conv"].tile([128, 2, TS], BF16, tag="conv")
        conv_t.append(cv)
        if korder == "center_first":
            pairs = [(0, 4), (1, 4)] + [(cc, k) for cc in (0, 1) for k in TAPS]
        else:
            pairs = [(cc, k) for cc in (0, 1) for k in range(K)]
        for oc in range(2):
            ps = P["ps"].tile([128, TS], F32, tag="ps")
            for idx, (cc, k) in enumerate(pairs):
                if nogate or k == 4:
                    rhs = xsrc[:, cc, k * DIL + tt * TS:
                               k * DIL + tt * TS + TS]
                elif pair_tm:
                    tmp, ii = tmul[k]
                    rhs = tmp[:, ii, cc, :]
                else:
                    rhs = tmul[k][:, cc, :]
                nc.tensor.matmul(
                    ps[:],
                    w_sb[:, k * 4 + cc * 2 + oc, :],
                    rhs,
                    start=(idx == 0),
                    stop=(idx == 17),
                )
            nc.scalar.activation(
                cv[:, oc, :], ps[:], AF.Copy,
                accum_out=s1cols[:, oc, t: t + 1],
            )
            sq = P["sq"].tile([128, TS], BF16, tag="sq")
            nc.scalar.activation(
                sq[:], ps[:], AF.Square,
                accum_out=s2cols[:, oc, t: t + 1],
            )

    # ---- phase 3: stats all-reduce + BN coefficients ----
    EE = getattr(nc, coef_eng)
    stats = P["small"].tile([128, 4], F32, tag="stats")
    nc.vector.tensor_reduce(stats[:, 0:2], s1cols[:], axis=mybir.AxisListType.X,
                            op=ALU.add)
    nc.vector.tensor_reduce(stats[:, 2:4], s2cols[:], axis=mybir.AxisListType.X,
                            op=ALU.add)
    cc_in = P["dram"].tile([128, 4], F32, tag="ccin")
    cc_out = P["dram"].tile([NUM_CORES * 128, 4], F32, tag="ccout")
    nc.gpsimd.dma_start(out=cc_in[:], in_=stats[:])
    red = P["small"].tile([128, 4], F32, tag="red")
    if single:
        nc.gpsimd.dma_start(out=cc_out[0:128, :], in_=cc_in[:])
        nc.gpsimd.dma_start(out=red[:], in_=cc_out[0:128, :])
    elif use_ar:
        nc.gpsimd.collective_compute(
            "AllReduce", ALU.add,
            replica_groups=[list(range(NUM_CORES))],
            ins=[cc_in.opt()], outs=[cc_out[0:128, :].opt()],
        )
        nc.gpsimd.dma_start(out=red[:], in_=cc_out[0:128, :])
    else:
        nc.gpsimd.collective_compute(
            "AllGather", ALU.bypass,
            replica_groups=[list(range(NUM_CORES))],
            ins=[cc_in.opt()], outs=[cc_out.opt()],
        )
        red8 = P["small"].tile([128, 4, NUM_CORES], F32, tag="red8")
        nc.gpsimd.dma_start(out=red8[:], in_=bass.AP(
            tensor=cc_out.tensor, offset=cc_out.offset,
            ap=[[4, 128], [1, 4], [512, NUM_CORES]],
        ))
        nc.vector.tensor_reduce(red[:], red8[:], axis=mybir.AxisListType.X,
                                op=ALU.add)

    me2 = P["small"].tile([128, 4], F32, tag="me2")
    EE.tensor_scalar_mul(me2[:], red[:], INV_COUNT)
    m = me2[:, 0:2]
    var = P["small"].tile([128, 2], F32, tag="var")
    EE.tensor_mul(var[:], m, m)
    EE.tensor_sub(var[:], me2[:, 2:4], var[:])
    EE.tensor_scalar_add(var[:], var[:], BN_EPS)
    # rinv = rsqrt(var+eps): bit-hack + Newton on DVE (no ACT table switch)
    vi = var[:].bitcast(mybir.dt.int32)
    yi = P["small"].tile([128, 2], mybir.dt.int32, tag="yi")
    EE.tensor_scalar(out=yi[:], in0=vi[:], scalar1=1, scalar2=None,
                            op0=ALU.arith_shift_right)
    EE.tensor_scalar(out=yi[:], in0=yi[:], scalar1=-1,
                            scalar2=0x5F3759DF, op0=ALU.mult, op1=ALU.add)
    rinv_t = P["small"].tile([128, 2], F32, tag="rinv")
    rinv = rinv_t[:]
    EE.tensor_copy(rinv, yi[:].bitcast(F32))
    hv = P["small"].tile([128, 2], F32, tag="hv")
    EE.tensor_scalar_mul(hv[:], var[:], -0.5)
    yy = P["small"].tile([128, 2], F32, tag="yy")
    for _ in range(2):
        EE.tensor_mul(yy[:], rinv, rinv)
        EE.tensor_mul(yy[:], yy[:], hv[:])
        EE.tensor_scalar_add(yy[:], yy[:], 1.5)
        EE.tensor_mul(rinv, rinv, yy[:])
    scl = P["small"].tile([128, 2], F32, tag="scl")
    EE.tensor_mul(scl[:], rinv, gam_sb[:])
    bia = P["small"].tile([128, 2], F32, tag="bia")
    EE.tensor_mul(bia[:], m, scl[:])
    EE.tensor_sub(bia[:], bet_sb[:], bia[:])

    # ---- phase 4: BN + ReLU + store (batched DMA on the tail queue) ----
    QT = qt  # tiles per output DMA
    for tq in range(NT // QT):
        for oc in range(2):
            fin = P["fin"].tile([128, QT, TS], F32, tag="fin")
            for j in range(QT):
                t = tq * QT + j
                use_scalar = (fin_mode == "scalar"
                              or (fin_mode in ("split", "splitp")
                                  and (2 * t + oc + j) % 2 == 0))
                if use_scalar:
                    nc.scalar.activation(
                        fin[:, j, :], conv_t[t][:, oc, :], AF.Relu,
                        bias=bia[:, oc: oc + 1], scale=scl[:, oc: oc + 1],
                    )
                elif fin_mode in ("pool", "splitp"):
                    def bv_(a, n=TS):
                        return bass.AP(tensor=a.tensor, offset=a.offset,
                                       ap=[a.ap[0], [0, n]])
                    nc.gpsimd.tensor_tensor(
                        out=fin[:, j, :], in0=conv_t[t][:, oc, :],
                        in1=bv_(scl[:, oc: oc + 1]), op=ALU.mult)
                    nc.gpsimd.tensor_tensor(
                        out=fin[:, j, :], in0=fin[:, j, :],
                        in1=bv_(bia[:, oc: oc + 1]), op=ALU.add)
                    nc.gpsimd.tensor_relu(fin[:, j, :], fin[:, j, :])
                else:
                    nc.vector.tensor_scalar(
                        out=fin[:, j, :], in0=conv_t[t][:, oc, :],
                        scalar1=scl[:, oc: oc + 1], scalar2=bia[:, oc: oc + 1],
                        op0=ALU.mult, op1=ALU.add,
                    )
                    nc.vector.tensor_scalar_max(fin[:, j, :], fin[:, j, :], 0.0)
            getattr(nc, store_eng).dma_start(
                out=out[oc * 128: (oc + 1) * 128,
                        tq * QT * TS: (tq + 1) * QT * TS],
                in_=fin[:].rearrange("p q t -> p (q t)"),
            )


def _prep_inputs(x, coords, rotations, distances, W, gamma, beta):
    """Host-side sharding/layout prep. Returns per-core input maps."""
    bf = ml_dtypes.bfloat16
    # weights: [o, c, k] -> 36 lhsT tiles [(k, cc, oc), c, o]
    wt = W.reshape(2, 128, 2, 128, K)            # [oc, o, cc, c, k]
    wt = wt.transpose(3, 4, 2, 0, 1)             # [c, k, cc, oc, o]
    wt = np.ascontiguousarray(wt.reshape(128, 36, 128), dtype=bf)
    gam2 = np.ascontiguousarray(gamma.reshape(2, 128), dtype=np.float32)
    bet2 = np.ascontiguousarray(beta.reshape(2, 128), dtype=np.float32)

    # gather index for the (half, tap, group)-blocked g layout
    ks = np.array(TAPS)
    idx = ((np.arange(2) * HGRP)[:, None, None, None]
           + (ks * DIL)[None, :, None, None]
           + (np.arange(NGRP) * GRP)[None, None, :, None]
           + np.arange(HGRP)[None, None, None, :])    # [2, KG, NGRP, HGRP]
    idx0 = idx - (ks * DIL)[None, :, None, None] + PAD
    in_maps = []
    for b in range(NUM_CORES):
        xpad = np.zeros((CIN, NXP), dtype=bf)
        xpad[:, PAD: PAD + N] = x[b].astype(bf)
        cd4 = np.zeros((4, NXP), dtype=np.float32)
        cd4[:3, PAD: PAD + N] = coords[b]
        cd4[3, PAD: PAD + N] = distances[b]
        rot = np.zeros((3, NXP), dtype=np.float32)
        rot[:, PAD: PAD + N] = rotations[b]
        cds_h = cd4[:, idx].transpose(1, 2, 3, 0, 4).reshape(128, 4, HGRP)
        cd0_h = cd4[:, idx0].transpose(1, 2, 3, 0, 4).reshape(128, 4, HGRP)
        r_s = rot[:, idx].transpose(1, 2, 3, 0, 4)    # [2, KG, NGRP, 3, HGRP]
        r_0 = rot[:, idx0].transpose(1, 2, 3, 0, 4)
        ral_h = np.stack([r_0, r_s], axis=3).reshape(128, 2, 3, HGRP)
        in_maps.append({
            "xp": np.ascontiguousarray(xpad.reshape(2, 128, NXP)),
            "cds": np.ascontiguousarray(cds_h.astype(np.float16)),
            "cd0": np.ascontiguousarray(cd0_h.astype(np.float16)),
            "ral": np.ascontiguousarray(ral_h.astype(np.float16)),
            "wt": wt,
            "gam": gam2,
            "bet": bet2,
        })
    return in_maps


def kernel(x, coords, rotations, distances, W, bias, gamma, beta):
    # accept jax arrays / array-likes as produced by reference.setup_inputs()
    x, coords, rotations, distances, W, gamma, beta = (
        np.asarray(a, dtype=np.float32)
        for a in (x, coords, rotations, distances, W, gamma, beta))
    if "nc" not in _CACHE:
        _CACHE["nc"] = _build_kernel()
    nc = _CACHE["nc"]
    in_maps = _prep_inputs(x, coords, rotations, distances, W, gamma, beta)
    res = run_bass_kernel_spmd(nc, in_maps, list(range(NUM_CORES)), trace=False)
    return np.stack([res.results[b]["out"] for b in range(NUM_CORES)], axis=0)

